# revision 37
# baseline (speedup 1.0000x reference)
"""EnhancedGCNII on 8 Trainium2 NeuronCores.

Strategy (row-sharded nodes, SBUF-resident transposed adjacency):
  - Host folds A+I into each core's row slab (part of normalize_adj), so
    A_hat @ M = dinv*((A+I) @ (dinv*M)) with deg = rowsum(A+I),
    dinv = rsqrt(deg). No separate self-term needed.
  - Associativity: a_hat @ (h @ W') = (a_hat @ h) @ W', so the SpMM only
    carries 128 features (P = dinv*H) instead of a 256-wide concat.
  - Core c owns node rows Rc = [c*1024, (c+1)*1024).
  - Pass 0: stream the 32MB fp32 adj row-slab from HBM once, cast to fp8
    (0/1/2 exact in fp8e4; degrees via accum_out on the scalar engine),
    transpose PAIRS of 128-row blocks per PE instruction using fp8
    DoubleRow matmuls against a packed selector. AT is stored (k, rb, s)
    so four transposes batch into one contiguous [128,1024] PSUM->SBUF
    copy. The dinv chain and the layer-0 gathers are emitted inside the
    pass-0 stream (after 7 of 8 slabs; the last slab's degree partial
    comes from a gpsimd reduce) so the first AllGathers overlap the
    pass-0 tail. A dummy AllGather at t=0 absorbs CC-stream warmup.
  - Per layer: transpose H -> node-major P = dinv*H (fp8, scaled on the
    scalar engine straight out of PSUM), AllGather P in two node-halves;
    gathers for layer i+1 are emitted inside layer i's epilogue halves.
    SpMM is fp8 DoubleRow, S^T = P^T @ A_loc^T.
    Epilogue: AH^T = dinv * S^T (bf16), then
      linear^T = W'^T AH^T + b',  gcnii^T = relu(M1^T AH^T + M0^T H0^T + b)
    with M1 = (1-a)((1-b)I + bW), M0 = a((1-b)I + bW), H_new = linear+gcnii.
  - Output: logits^T = fc_out_w^T @ h^T computed locally, host transposes
    and concatenates.
"""

import sys
import types

sys.path.insert(0, "/opt/trn_rl_repo")

# ---------------------------------------------------------------------------
# Environment shims (axon container):
#  - antenv.axon_hooks is absent; register the NTFF profile hook ourselves so
#    trace=True yields exec_time_ns.
#  - no artifact bucket; skip uploads.
#  - walrus in this container allows only ONE semaphore wait on the CTRL
#    instruction Tile emits as the kernel-tail drain; split the waits across
#    sequential NOPs.
# ---------------------------------------------------------------------------
import antenv  # noqa: E402

if "antenv.axon_hooks" not in sys.modules:
    _mod = types.ModuleType("antenv.axon_hooks")
    _hook = [None]
    _mod.set_axon_ntff_profile_hook = lambda h: _hook.__setitem__(0, h)
    _mod.get_axon_ntff_profile_hook = lambda: _hook[0]
    sys.modules["antenv.axon_hooks"] = _mod
    antenv.axon_hooks = _mod
    try:
        from trn_agent_boot.trn_boot import _ntff_profile_via_ctypes

        _mod.set_axon_ntff_profile_hook(
            _ntff_profile_via_ctypes("/opt/axon/libaxon_pjrt.so")
        )
    except Exception as _e:
        print(f"ntff hook registration failed: {_e}", file=sys.stderr)

import numpy as np  # noqa: E402
import ml_dtypes  # noqa: E402
import concourse.bass as bass  # noqa: E402
import concourse.bacc as bacc  # noqa: E402
import concourse.mybir as mybir  # noqa: E402
import concourse.tile as tile  # noqa: E402
from concourse import bass_utils  # noqa: E402

bass_utils.upload_artifacts = lambda tmpdir: f"local://{tmpdir}"

_MAX_DRAIN_WAITS = 1


def _split_drain_and_barrier(self, tick_clock, wait_clock):
    nc = self.nc
    carrier = nc.sync.nop(hint="drain_wait_carrier", nofuse=True)
    wait_clock.add_sem_waits(
        carrier.ins, tile.ScopedClock({None: tick_clock.global_clock})
    )
    si = carrier.ins.sync_info
    if si is not None and len(si.on_wait) > _MAX_DRAIN_WAITS:
        waits = list(si.on_wait)
        carrier.ins.sync_info = mybir.SyncInfo(
            on_wait=waits[:_MAX_DRAIN_WAITS], on_update=list(si.on_update)
        )
        for i in range(_MAX_DRAIN_WAITS, len(waits), _MAX_DRAIN_WAITS):
            extra = nc.sync.nop(hint="drain_wait_split", nofuse=True)
            extra.ins.sync_info = mybir.SyncInfo(
                on_wait=waits[i : i + _MAX_DRAIN_WAITS], on_update=[]
            )
    nc.sync.drain()
    nc.all_engine_barrier()
    assert self.sems is not None
    popped = nc._tile_sem_poison_stack.pop()
    assert popped is self._sem_poison
    nc.clear_and_free_semaphores(list(self.sems.allocated().values()))
    nc.all_engine_barrier()


tile.TileContext._drain_and_barrier = _split_drain_and_barrier

# ---------------------------------------------------------------------------
# Problem constants (hardcoded per the harness contract)
# ---------------------------------------------------------------------------
import math  # noqa: E402

N, NFEAT, NHID, NCLASS, NLAYERS = 8192, 500, 128, 40, 4
ALPHA, GAMMA, LAMBDA = 0.1, 0.1, 0.5
NCORES = 8
NLOC = N // NCORES  # 1024 local nodes per core
K = N // 128  # 64 node chunks
KP = K // 2  # 32 chunk pairs (DoubleRow)
RB = NLOC // 128  # 8 local row blocks
NFP = 512  # padded feature dim

F32 = mybir.dt.float32
BF16 = mybir.dt.bfloat16
FP8 = mybir.dt.float8e4


def build_program():
    nc = bacc.Bacc(num_devices=NCORES)

    adj_c = nc.dram_tensor("adj_c", [NLOC, N], F32, kind="ExternalInput")
    x_c = nc.dram_tensor("x_c", [NLOC, NFP], F32, kind="ExternalInput")
    fcw_d = nc.dram_tensor("fc_in_w_p", [NFP, NHID], F32, kind="ExternalInput")
    fcb_d = nc.dram_tensor("fc_in_b", [NHID], F32, kind="ExternalInput")
    c_d = nc.dram_tensor("c_vec", [NHID], F32, kind="ExternalInput")
    wg_d = nc.dram_tensor("w_gcnii", [NLAYERS, NHID, NHID], F32, kind="ExternalInput")
    bg_d = nc.dram_tensor("b_gcnii", [NLAYERS, NHID], F32, kind="ExternalInput")
    wl_d = nc.dram_tensor("w_lin", [NLAYERS, NHID, NHID], F32, kind="ExternalInput")
    bl_d = nc.dram_tensor("b_lin", [NLAYERS, NHID], F32, kind="ExternalInput")
    fow_d = nc.dram_tensor("fc_out_w", [NHID, NCLASS], F32, kind="ExternalInput")
    fob_d = nc.dram_tensor("fc_out_b", [NCLASS], F32, kind="ExternalInput")
    out_t = nc.dram_tensor("out_t", [NCLASS, NLOC], F32, kind="ExternalOutput")

    ident_d = nc.inline_tensor(np.eye(128, dtype=np.float32), name="ident128")

    betas = [math.log(LAMBDA / (i + 1) + 1.0) for i in range(NLAYERS)]

    with tile.TileContext(nc, num_cores=NCORES) as tc:
        with (
            tc.tile_pool(name="persist", bufs=1) as pp,
            tc.tile_pool(name="state", bufs=2) as stp,
            tc.tile_pool(name="dram", bufs=1, space="DRAM") as dram,
        ):
            # ---- CC warmup: absorb collective-stream setup under pass 0 ----
            warm_in = dram.tile([8, 4], F32, name="warm_in")
            warm_out = dram.tile([64, 4], F32, addr_space="Shared", name="warm_out")
            nc.gpsimd.collective_compute(
                "AllGather",
                mybir.AluOpType.bypass,
                replica_groups=[list(range(NCORES))],
                ins=[warm_in[:].opt()],
                outs=[warm_out[:].opt()],
            )

            # ---- persistent SBUF tiles ----
            at_all = pp.tile([128, K * RB * 128], FP8)  # 64KB/partition
            ident = pp.tile([128, 128], F32)
            nc.sync.dma_start(ident[:], ident_d[:])
            ones_row = pp.tile([1, 128], F32)
            nc.vector.memset(ones_row[:], 1.0)
            ident_bf = pp.tile([128, 128], BF16)
            nc.vector.tensor_copy(ident_bf[:], ident[:])
            ident_f8 = pp.tile([128, 128], FP8)
            nc.vector.tensor_copy(ident_f8[:], ident[:])
            # packed DoubleRow transpose selector: sel[p, o, h*128+y] = (h==o && y==p)
            sel2 = pp.tile([128, 512], FP8)
            nc.vector.memset(sel2[:], 0.0)
            sel2_v = sel2[:].rearrange("p (o h y) -> p o h y", o=2, h=2)
            nc.vector.tensor_copy(sel2_v[:, 0, 0, :], ident_f8[:])
            nc.vector.tensor_copy(sel2_v[:, 1, 1, :], ident_f8[:])

            wg_sb = pp.tile([128, NLAYERS * 128], F32)
            nc.sync.dma_start(
                wg_sb[:].rearrange("p (l f) -> p l f", l=NLAYERS),
                wg_d[:].rearrange("l p f -> p l f"),
            )
            wl_sb = pp.tile([128, NLAYERS * 128], F32)
            nc.sync.dma_start(
                wl_sb[:].rearrange("p (l f) -> p l f", l=NLAYERS),
                wl_d[:].rearrange("l p f -> p l f"),
            )
            bg_sb = pp.tile([128, NLAYERS], F32)
            nc.sync.dma_start(bg_sb[:], bg_d[:].rearrange("l p -> p l"))
            bl_sb = pp.tile([128, NLAYERS], F32)
            nc.sync.dma_start(bl_sb[:], bl_d[:].rearrange("l p -> p l"))
            fcw_sb = pp.tile([128, 4 * 128], F32)
            nc.sync.dma_start(
                fcw_sb[:].rearrange("p (j f) -> p j f", j=4),
                fcw_d[:].rearrange("(j p) f -> p j f", p=128),
            )
            fcb_sb = pp.tile([128, 1], F32)
            nc.sync.dma_start(fcb_sb[:], fcb_d[:].rearrange("(p o) -> p o", o=1))
            c_sb = pp.tile([128, 1], F32)
            nc.sync.dma_start(c_sb[:], c_d[:].rearrange("(p o) -> p o", o=1))
            fow_sb = pp.tile([128, NCLASS], F32)
            nc.sync.dma_start(fow_sb[:], fow_d[:])
            fob_sb = pp.tile([NCLASS, 1], F32)
            nc.sync.dma_start(fob_sb[:], fob_d[:].rearrange("(p o) -> p o", o=1))

            c01 = pp.tile([128, 1], F32)
            nc.vector.tensor_scalar_mul(c01[:], c_sb[:], GAMMA)
            fcw_bf = pp.tile([128, 4 * 128], BF16)
            nc.vector.tensor_copy(fcw_bf[:], fcw_sb[:])

            # M_i = beta_i*wg_i + (1-beta_i)*I; m1 = (1-a)M, m0 = a*M (bf16)
            m1_bf = pp.tile([128, NLAYERS * 128], BF16)
            m0_bf = pp.tile([128, NLAYERS * 128], BF16)
            wl_bf = pp.tile([128, NLAYERS * 128], BF16)
            nc.vector.tensor_copy(wl_bf[:], wl_sb[:])
            for i in range(NLAYERS):
                sl = slice(i * 128, (i + 1) * 128)
                mtmp = stp.tile([128, 128], F32, tag="mtmp")
                nc.vector.tensor_scalar_mul(mtmp[:], wg_sb[:, sl], betas[i])
                mtmp2 = stp.tile([128, 128], F32, tag="mtmp2")
                nc.vector.tensor_scalar_mul(mtmp2[:], ident[:], 1.0 - betas[i])
                nc.vector.tensor_add(mtmp[:], mtmp[:], mtmp2[:])
                nc.vector.tensor_scalar_mul(m1_bf[:, sl], mtmp[:], 1.0 - ALPHA)
                nc.vector.tensor_scalar_mul(m0_bf[:, sl], mtmp[:], ALPHA)

            fow_bf = pp.tile([128, NCLASS], BF16)
            nc.vector.tensor_copy(fow_bf[:], fow_sb[:])

            dinv_row = pp.tile([1, NLOC], F32)
            dinv_nch = pp.tile([128, RB], F32)
            h0_bf = pp.tile([128, NLOC], BF16)

            # transposed-A views: free index = rb*8192 + k*128 + s
            at_dr = at_all[:].rearrange(
                "p (rb kp o s) -> p rb kp o s", rb=RB, kp=KP, o=2
            )

            # =============== fc_in (x -> h0^T, bf16) ========================
            def emit_fc():
              with (
                tc.tile_pool(name="fcpool", bufs=2) as fcp,
                tc.tile_pool(name="ps_fc", bufs=2, space="PSUM") as psfc,
              ):
                xt_all = fcp.tile([128, 4 * NLOC], BF16, bufs=1)
                for rb in range(RB):
                    x_chunk = fcp.tile([128, NFP], F32, tag="xch")
                    nc.sync.dma_start(
                        x_chunk[:], x_c[rb * 128 : (rb + 1) * 128, :]
                    )
                    x_stage = fcp.tile([128, NFP], BF16, tag="xbf")
                    nc.vector.tensor_copy(x_stage[:], x_chunk[:])
                    ps_x = psfc.tile([128, 512], F32, tag="psfc")
                    for j in range(4):
                        nc.tensor.matmul(
                            ps_x[:, j * 128 : (j + 1) * 128],
                            x_stage[:, j * 128 : (j + 1) * 128],
                            ident_bf[:],
                            start=(j == 0),
                            stop=(j == 3),
                            skip_group_check=True,
                        )
                    xt_view = xt_all[:].rearrange(
                        "p (j rb s) -> p j rb s", j=4, rb=RB, s=128
                    )
                    nc.vector.tensor_copy(xt_view[:, :, rb : rb + 1, :], ps_x[:])
                for nh in range(2):
                    ps_h = psfc.tile([128, 512], F32, tag="psfc")
                    for j in range(4):
                        nc.tensor.matmul(
                            ps_h[:],
                            fcw_bf[:, j * 128 : (j + 1) * 128],
                            xt_all[:, j * NLOC + nh * 512 : j * NLOC + (nh + 1) * 512],
                            start=(j == 0),
                            stop=(j == 3),
                        )
                    htmp = fcp.tile([128, 512], F32, tag="htmp")
                    nc.scalar.activation(
                        htmp[:],
                        ps_h[:],
                        mybir.ActivationFunctionType.Relu,
                        bias=fcb_sb[:, 0:1],
                    )
                    nc.scalar.activation(
                        h0_bf[:, nh * 512 : (nh + 1) * 512],
                        htmp[:],
                        mybir.ActivationFunctionType.Identity,
                        bias=c01[:, 0:1],
                        scale=1.0 - GAMMA,
                    )

            # ====== pass 0 + layers share pools (layer-0 gathers are emitted
            # inside the pass-0 stream) ======
            with (
                tc.tile_pool(name="apool", bufs=2) as ap_pool,
                tc.tile_pool(name="lpool", bufs=1) as lp,
                tc.tile_pool(name="tmp4", bufs=4) as tp,
                tc.tile_pool(name="ps_p", bufs=2, space="PSUM") as ps_pp,
            ):
                deg_parts = pp.tile([128, 32], F32)
                deg_nch = pp.tile([128, RB], F32)
                psb = [
                    [
                        lp.tile([128, 32 * 128], FP8, name=f"psb{par}_{h}")
                        for h in range(2)
                    ]
                    for par in range(2)
                ]
                plocs = [
                    tp.tile([128, RB * 128], FP8, tag=f"ploc{i % 2}", bufs=1,
                            name=f"ploc{i}")
                    for i in range(NLAYERS)
                ]

                def emit_gather(i, half, hT_src, dq=None):
                    """transpose H half -> node-major fp8 P = dinv*H (scaled
                    on scalar straight out of PSUM), AllGather it, land it in
                    psb[i%2][half]. dq picks the DMA queue for cin/psb moves
                    (gpsimd during pass 0, idle sync queue inside layers)."""
                    dq = dq or nc.gpsimd
                    ploc = plocs[i]
                    cin = dram.tile([NLOC // 2, 128], FP8, name=f"ccin{i}_{half}")
                    cout = dram.tile(
                        [N // 2, 128], FP8, addr_space="Shared",
                        name=f"ccout{i}_{half}",
                    )
                    for nb in range(half * 4, half * 4 + 4):
                        ps_p = ps_pp.tile([128, 128], F32, tag="psp")
                        nc.tensor.matmul(
                            ps_p[:],
                            hT_src[:, nb * 128 : (nb + 1) * 128],
                            ident_bf[:],
                            start=True,
                            stop=True,
                        )
                        nc.scalar.activation(
                            ploc[:, nb * 128 : (nb + 1) * 128],
                            ps_p[:],
                            mybir.ActivationFunctionType.Copy,
                            scale=dinv_nch[:, nb : nb + 1],
                        )
                    dq.dma_start(
                        cin[:].rearrange("(nb p) f -> p nb f", p=128),
                        ploc[:, half * 512 : (half + 1) * 512].rearrange(
                            "p (nb f) -> p nb f", nb=4
                        ),
                    )
                    nc.gpsimd.collective_compute(
                        "AllGather",
                        mybir.AluOpType.bypass,
                        replica_groups=[list(range(NCORES))],
                        ins=[cin[:].opt()],
                        outs=[cout[:].opt()],
                    )
                    dq.dma_start(
                        psb[i % 2][half][:].rearrange(
                            "p (c q f) -> p c q f", c=8, q=4
                        ),
                        cout[:].rearrange("(c q p) f -> p c q f", p=128, q=4),
                    )

                # ---- pass 0: 16 slabs of (2 row-blocks x 2048 cols) ----
                # (rbp-major order: after all four column-quarters of a
                # row-block pair, that pair's degrees are final -> dinv)
                CW = 2048  # slab column width
                NCC = N // CW  # 4 column quarters
                slabs = [(cc, rbp) for rbp in range(RB // 2) for cc in range(NCC)]
                a_f8s = {}

                def slab_load(idx):
                    cc, rbp = slabs[idx]
                    a_pair = ap_pool.tile([128, 2 * CW], F32, tag="apair")
                    nc.sync.dma_start(
                        a_pair[:].rearrange("p (o c) -> p o c", o=2),
                        adj_c[
                            rbp * 256 : (rbp + 1) * 256,
                            cc * CW : (cc + 1) * CW,
                        ].rearrange("(o p) c -> p o c", p=128),
                    )
                    a_bf = ap_pool.tile([128, 2 * CW], BF16, tag="abf")
                    for o in range(2):
                        dix = (rbp * 2 + o) * NCC + cc
                        nc.scalar.activation(
                            a_bf[:, o * CW : (o + 1) * CW],
                            a_pair[:, o * CW : (o + 1) * CW],
                            mybir.ActivationFunctionType.Copy,
                            accum_out=deg_parts[:, dix : dix + 1],
                        )
                    a_f8s[idx] = a_bf

                def slab_transpose(idx):
                    """XBAR-transpose each row-block's strip on the scalar
                    HWDGE ring, then recast bf16 -> fp8 into at_all (both
                    sides contiguous)."""
                    cc, rbp = slabs[idx]
                    a_bf = a_f8s.pop(idx)
                    for o in range(2):
                        rb = rbp * 2 + o
                        stage = ap_pool.tile([128, CW], BF16, tag="stage", bufs=2)
                        nc.scalar.dma_start(
                            stage[:].rearrange("p (k s) -> p k s", s=128),
                            a_bf[:, o * CW : (o + 1) * CW],
                            transpose=True,
                        )
                        dst = at_all[
                            :, rb * 8192 + cc * CW : rb * 8192 + (cc + 1) * CW
                        ]
                        if idx % 4 == 3:
                            nc.scalar.copy(dst, stage[:])
                        else:
                            nc.vector.tensor_copy(dst, stage[:])

                with tc.tile_pool(name="ps_tr", bufs=2, space="PSUM") as ps_trp:
                    rec_nch = pp.tile([128, RB], F32)
                    deg_rbc = deg_parts[:].rearrange("p (rb cc) -> p rb cc", cc=NCC)
                    for rbp in range(RB // 2):
                        for cc in range(NCC):
                            idx = rbp * NCC + cc
                            slab_load(idx)
                            if idx == 0:
                                slab_transpose(0)
                                # fc_in emitted here: its x DMA queues behind
                                # slab 0, its PE work fills the slab-1 wait
                                emit_fc()
                                continue
                            slab_transpose(idx)
                        # degrees for row blocks (2rbp, 2rbp+1) are final now:
                        # their full 8192 columns have been accumulated.
                        dsl = slice(2 * rbp, 2 * rbp + 2)
                        nc.vector.tensor_reduce(
                            deg_nch[:, dsl],
                            deg_rbc[:, dsl, :],
                            axis=mybir.AxisListType.X,
                            op=mybir.AluOpType.add,
                        )
                        nc.vector.reciprocal(rec_nch[:, dsl], deg_nch[:, dsl])
                        nc.scalar.sqrt(dinv_nch[:, dsl], rec_nch[:, dsl])
                        if rbp == 1:
                            emit_gather(0, 0, h0_bf)
                        elif rbp == 3:
                            dinv_dram = dram.tile([1, NLOC], F32, name="dinv_dram")
                            nc.sync.dma_start(
                                dinv_dram[:].rearrange("o (j p) -> (o p) j", p=128),
                                dinv_nch[:],
                            )
                            nc.sync.dma_start(dinv_row[:], dinv_dram[:])
                            emit_gather(0, 1, h0_bf)

                # ---- layers ----
                with (
                    tc.tile_pool(name="ps_st", bufs=1, space="PSUM") as ps_stp,
                    tc.tile_pool(name="ps_aux", bufs=2, space="PSUM") as ps_auxp,
                ):
                    b_d1 = lp.tile([128, NLOC], F32)
                    ps_b = ps_auxp.tile([128, 512], F32, tag="aux", name="psb_bc0")
                    ps_b2 = ps_auxp.tile([128, 512], F32, tag="aux", name="psb_bc1")
                    for nh, psx in ((0, ps_b), (1, ps_b2)):
                        nc.tensor.matmul(
                            psx[:],
                            ones_row[0:1, :],
                            dinv_row[0:1, nh * 512 : (nh + 1) * 512],
                            start=True,
                            stop=True,
                        )
                        nc.vector.tensor_copy(
                            b_d1[:, nh * 512 : (nh + 1) * 512], psx[:]
                        )

                    hT = h0_bf
                    for i in range(NLAYERS):
                        # ---- SpMM: S^T = P^T @ A_loc^T (fp8 DoubleRow) ----
                        # half h holds global chunks kk = c*8 + h*4 + {0..3},
                        # i.e. chunk pairs kp = c*4 + h*2 + {0,1}.
                        # rh-major SpMM: st[:, 0:512] finishes at half-SpMM so
                        # its epilogue + the next-layer gather overlap rh=1.
                        st = ps_stp.tile([128, NLOC], F32, tag="st", name=f"st_{i}")
                        ah_bf = tp.tile(
                            [128, NLOC], BF16, tag="ahbf", bufs=2, name=f"ah{i}"
                        )
                        hT_new = stp.tile(
                            [128, NLOC], BF16, tag="hT", name=f"hT_l{i + 1}"
                        )
                        for nh in range(2):
                            rh = nh
                            n_mm = 0
                            for half in range(2):
                                psb_v = psb[i % 2][half][:].rearrange(
                                    "p (c j o f) -> p c j o f", c=8, j=2, o=2
                                )
                                for c in range(8):
                                    for j in range(2):
                                        kp = c * 4 + half * 2 + j
                                        lhs_dr = psb_v[:, c, j, :, :]
                                        rhs_dr = at_dr[
                                            :, rh * 4 : (rh + 1) * 4, kp, :, :
                                        ].rearrange("p rb o s -> p o rb s")
                                        nc.tensor.matmul(
                                            st[:, rh * 512 : (rh + 1) * 512],
                                            lhs_dr,
                                            rhs_dr,
                                            start=(n_mm == 0),
                                            stop=(n_mm == KP - 1),
                                            perf_mode=mybir.MatmulPerfMode.DoubleRow,
                                        )
                                        n_mm += 1

                            sl = slice(nh * 512, (nh + 1) * 512)
                            nc.vector.tensor_mul(ah_bf[:, sl], st[:, sl], b_d1[:, sl])
                            ps_l = ps_auxp.tile(
                                [128, 512], F32, tag="aux", name=f"psl{i}_{nh}"
                            )
                            nc.tensor.matmul(
                                ps_l[:],
                                wl_bf[:, i * 128 : (i + 1) * 128],
                                ah_bf[:, sl],
                                start=True,
                                stop=True,
                            )
                            ps_g = ps_auxp.tile(
                                [128, 512], F32, tag="aux", name=f"psg{i}_{nh}"
                            )
                            nc.tensor.matmul(
                                ps_g[:],
                                m1_bf[:, i * 128 : (i + 1) * 128],
                                ah_bf[:, sl],
                                start=True,
                                stop=False,
                            )
                            nc.tensor.matmul(
                                ps_g[:],
                                m0_bf[:, i * 128 : (i + 1) * 128],
                                h0_bf[:, sl],
                                start=False,
                                stop=True,
                            )
                            linv = tp.tile(
                                [128, 512], F32, tag="linv", bufs=2,
                                name=f"lv{i}_{nh}",
                            )
                            nc.scalar.activation(
                                linv[:],
                                ps_l[:],
                                mybir.ActivationFunctionType.Identity,
                                bias=bl_sb[:, i : i + 1],
                            )
                            gc = tp.tile(
                                [128, 512], F32, tag="gc", bufs=2,
                                name=f"gc{i}_{nh}",
                            )
                            nc.scalar.activation(
                                gc[:],
                                ps_g[:],
                                mybir.ActivationFunctionType.Relu,
                                bias=bg_sb[:, i : i + 1],
                            )
                            nc.vector.tensor_add(hT_new[:, sl], linv[:], gc[:])
                            if i < NLAYERS - 1:
                                emit_gather(i + 1, nh, hT_new, dq=nc.sync)
                        hT = hT_new

                    # ---- output head ----
                    ps_o = ps_auxp.tile(
                        [NCLASS, NLOC], F32, tag="auxo", name="pso", bufs=1
                    )
                    for nh in range(2):
                        nc.tensor.matmul(
                            ps_o[:, nh * 512 : (nh + 1) * 512],
                            fow_bf[:, 0:NCLASS],
                            hT[:, nh * 512 : (nh + 1) * 512],
                            start=True,
                            stop=True,
                        )
                    out_sb = lp.tile([NCLASS, NLOC], F32)
                    nc.scalar.activation(
                        out_sb[:],
                        ps_o[:],
                        mybir.ActivationFunctionType.Identity,
                        bias=fob_sb[:, 0:1],
                    )
                    nc.sync.dma_start(out_t[:], out_sb[:])

    nc.compile()
    return nc


_program_cache = {}


def _get_program():
    if "nc" not in _program_cache:
        _program_cache["nc"] = build_program()
    return _program_cache["nc"]


def kernel(
    x,
    adj,
    fc_in_w,
    fc_in_b,
    c,
    w_gcnii,
    b_gcnii,
    w_lin,
    b_lin,
    fc_out_w,
    fc_out_b,
    _trace=False,
):
    x = np.asarray(x, dtype=np.float32)
    adj = np.asarray(adj, dtype=np.float32)
    x_pad = np.zeros((N, NFP), np.float32)
    x_pad[:, :NFEAT] = x
    fcw_pad = np.zeros((NFP, NHID), np.float32)
    fcw_pad[:NFEAT, :] = np.asarray(fc_in_w, np.float32)

    shared = {
        "fc_in_w_p": fcw_pad,
        "fc_in_b": np.asarray(fc_in_b, np.float32),
        "c_vec": np.asarray(c, np.float32),
        "w_gcnii": np.ascontiguousarray(w_gcnii, np.float32),
        "b_gcnii": np.ascontiguousarray(b_gcnii, np.float32),
        "w_lin": np.ascontiguousarray(w_lin, np.float32),
        "b_lin": np.ascontiguousarray(b_lin, np.float32),
        "fc_out_w": np.ascontiguousarray(fc_out_w, np.float32),
        "fc_out_b": np.asarray(fc_out_b, np.float32),
    }
    rr = np.arange(NLOC)
    in_maps = []
    for cix in range(NCORES):
        r0, r1 = cix * NLOC, (cix + 1) * NLOC
        m = dict(shared)
        blk = np.array(adj[r0:r1, :], dtype=np.float32, copy=True)
        blk[rr, r0 + rr] += 1.0  # fold A+I during sharding
        m["adj_c"] = blk
        m["x_c"] = np.ascontiguousarray(x_pad[r0:r1, :])
        in_maps.append(m)

    nc = _get_program()
    res = bass_utils.run_bass_kernel_spmd(
        nc, in_maps=in_maps, core_ids=list(range(NCORES)), trace=_trace
    )
    out = np.empty((N, NCLASS), np.float32)
    for cix in range(NCORES):
        out[cix * NLOC : (cix + 1) * NLOC, :] = res.results[cix]["out_t"].T
    kernel.last_exec_time_ns = res.exec_time_ns
    kernel.last_results = res
    return out


kernel.last_exec_time_ns = None
kernel.last_results = None


# revision 40
# speedup vs baseline: 1.5020x; 1.5020x over previous
"""EnhancedGCNII on 8 Trainium2 NeuronCores.

Strategy (row-sharded nodes, SBUF-resident transposed adjacency):
  - Host folds A+I into each core's row slab (part of normalize_adj), so
    A_hat @ M = dinv*((A+I) @ (dinv*M)) with deg = rowsum(A+I),
    dinv = rsqrt(deg). No separate self-term needed.
  - Associativity: a_hat @ (h @ W') = (a_hat @ h) @ W', so the SpMM only
    carries 128 features (P = dinv*H) instead of a 256-wide concat.
  - Core c owns node rows Rc = [c*1024, (c+1)*1024).
  - Pass 0: stream the 32MB fp32 adj row-slab from HBM once, cast to fp8
    (0/1/2 exact in fp8e4; degrees via accum_out on the scalar engine),
    transpose PAIRS of 128-row blocks per PE instruction using fp8
    DoubleRow matmuls against a packed selector. AT is stored (k, rb, s)
    so four transposes batch into one contiguous [128,1024] PSUM->SBUF
    copy. The dinv chain and the layer-0 gathers are emitted inside the
    pass-0 stream (after 7 of 8 slabs; the last slab's degree partial
    comes from a gpsimd reduce) so the first AllGathers overlap the
    pass-0 tail. A dummy AllGather at t=0 absorbs CC-stream warmup.
  - Per layer: transpose H -> node-major P = dinv*H (fp8, scaled on the
    scalar engine straight out of PSUM), AllGather P in two node-halves;
    gathers for layer i+1 are emitted inside layer i's epilogue halves.
    SpMM is fp8 DoubleRow, S^T = P^T @ A_loc^T.
    Epilogue: AH^T = dinv * S^T (bf16), then
      linear^T = W'^T AH^T + b',  gcnii^T = relu(M1^T AH^T + M0^T H0^T + b)
    with M1 = (1-a)((1-b)I + bW), M0 = a((1-b)I + bW), H_new = linear+gcnii.
  - Output: logits^T = fc_out_w^T @ h^T computed locally, host transposes
    and concatenates.
"""

import sys
import types

sys.path.insert(0, "/opt/trn_rl_repo")

# ---------------------------------------------------------------------------
# Environment shims (axon container):
#  - antenv.axon_hooks is absent; register the NTFF profile hook ourselves so
#    trace=True yields exec_time_ns.
#  - no artifact bucket; skip uploads.
#  - walrus in this container allows only ONE semaphore wait on the CTRL
#    instruction Tile emits as the kernel-tail drain; split the waits across
#    sequential NOPs.
# ---------------------------------------------------------------------------
import antenv  # noqa: E402

if "antenv.axon_hooks" not in sys.modules:
    _mod = types.ModuleType("antenv.axon_hooks")
    _hook = [None]
    _mod.set_axon_ntff_profile_hook = lambda h: _hook.__setitem__(0, h)
    _mod.get_axon_ntff_profile_hook = lambda: _hook[0]
    sys.modules["antenv.axon_hooks"] = _mod
    antenv.axon_hooks = _mod
    try:
        from trn_agent_boot.trn_boot import _ntff_profile_via_ctypes

        _mod.set_axon_ntff_profile_hook(
            _ntff_profile_via_ctypes("/opt/axon/libaxon_pjrt.so")
        )
    except Exception as _e:
        print(f"ntff hook registration failed: {_e}", file=sys.stderr)

import numpy as np  # noqa: E402
import ml_dtypes  # noqa: E402
import concourse.bass as bass  # noqa: E402
import concourse.bacc as bacc  # noqa: E402
import concourse.mybir as mybir  # noqa: E402
import concourse.tile as tile  # noqa: E402
from concourse import bass_utils  # noqa: E402

bass_utils.upload_artifacts = lambda tmpdir: f"local://{tmpdir}"

_MAX_DRAIN_WAITS = 1


def _split_drain_and_barrier(self, tick_clock, wait_clock):
    nc = self.nc
    carrier = nc.sync.nop(hint="drain_wait_carrier", nofuse=True)
    wait_clock.add_sem_waits(
        carrier.ins, tile.ScopedClock({None: tick_clock.global_clock})
    )
    si = carrier.ins.sync_info
    if si is not None and len(si.on_wait) > _MAX_DRAIN_WAITS:
        waits = list(si.on_wait)
        carrier.ins.sync_info = mybir.SyncInfo(
            on_wait=waits[:_MAX_DRAIN_WAITS], on_update=list(si.on_update)
        )
        for i in range(_MAX_DRAIN_WAITS, len(waits), _MAX_DRAIN_WAITS):
            extra = nc.sync.nop(hint="drain_wait_split", nofuse=True)
            extra.ins.sync_info = mybir.SyncInfo(
                on_wait=waits[i : i + _MAX_DRAIN_WAITS], on_update=[]
            )
    nc.sync.drain()
    nc.all_engine_barrier()
    assert self.sems is not None
    popped = nc._tile_sem_poison_stack.pop()
    assert popped is self._sem_poison
    nc.clear_and_free_semaphores(list(self.sems.allocated().values()))
    nc.all_engine_barrier()


tile.TileContext._drain_and_barrier = _split_drain_and_barrier

# ---------------------------------------------------------------------------
# Problem constants (hardcoded per the harness contract)
# ---------------------------------------------------------------------------
import math  # noqa: E402

N, NFEAT, NHID, NCLASS, NLAYERS = 8192, 500, 128, 40, 4
ALPHA, GAMMA, LAMBDA = 0.1, 0.1, 0.5
NCORES = 8
NLOC = N // NCORES  # 1024 local nodes per core
K = N // 128  # 64 node chunks
KP = K // 2  # 32 chunk pairs (DoubleRow)
RB = NLOC // 128  # 8 local row blocks
NFP = 512  # padded feature dim

F32 = mybir.dt.float32
BF16 = mybir.dt.bfloat16
FP8 = mybir.dt.float8e4


def build_program():
    nc = bacc.Bacc(num_devices=NCORES)

    adj_c = nc.dram_tensor("adj_c", [NLOC, N], F32, kind="ExternalInput")
    x_c = nc.dram_tensor("x_c", [NLOC, NFP], F32, kind="ExternalInput")
    fcw_d = nc.dram_tensor("fc_in_w_p", [NFP, NHID], F32, kind="ExternalInput")
    fcb_d = nc.dram_tensor("fc_in_b", [NHID], F32, kind="ExternalInput")
    c_d = nc.dram_tensor("c_vec", [NHID], F32, kind="ExternalInput")
    wg_d = nc.dram_tensor("w_gcnii", [NLAYERS, NHID, NHID], F32, kind="ExternalInput")
    bg_d = nc.dram_tensor("b_gcnii", [NLAYERS, NHID], F32, kind="ExternalInput")
    wl_d = nc.dram_tensor("w_lin", [NLAYERS, NHID, NHID], F32, kind="ExternalInput")
    bl_d = nc.dram_tensor("b_lin", [NLAYERS, NHID], F32, kind="ExternalInput")
    fow_d = nc.dram_tensor("fc_out_w", [NHID, NCLASS], F32, kind="ExternalInput")
    fob_d = nc.dram_tensor("fc_out_b", [NCLASS], F32, kind="ExternalInput")
    out_t = nc.dram_tensor("out_t", [NCLASS, NLOC], F32, kind="ExternalOutput")

    ident_d = nc.inline_tensor(np.eye(128, dtype=np.float32), name="ident128")

    betas = [math.log(LAMBDA / (i + 1) + 1.0) for i in range(NLAYERS)]

    with tile.TileContext(nc, num_cores=NCORES) as tc:
        with (
            tc.tile_pool(name="persist", bufs=1) as pp,
            tc.tile_pool(name="state", bufs=2) as stp,
            tc.tile_pool(name="dram", bufs=1, space="DRAM") as dram,
        ):
            # ---- CC warmup: absorb collective-stream setup under pass 0 ----
            warm_in = dram.tile([8, 4], F32, name="warm_in")
            warm_out = dram.tile([64, 4], F32, addr_space="Shared", name="warm_out")
            nc.gpsimd.collective_compute(
                "AllGather",
                mybir.AluOpType.bypass,
                replica_groups=[list(range(NCORES))],
                ins=[warm_in[:].opt()],
                outs=[warm_out[:].opt()],
            )

            # ---- persistent SBUF tiles ----
            at_all = pp.tile([128, K * RB * 128], FP8)  # 64KB/partition
            ident = pp.tile([128, 128], F32)
            nc.sync.dma_start(ident[:], ident_d[:])
            ones_row = pp.tile([1, 128], F32)
            nc.vector.memset(ones_row[:], 1.0)
            ident_bf = pp.tile([128, 128], BF16)
            nc.vector.tensor_copy(ident_bf[:], ident[:])
            ident_f8 = pp.tile([128, 128], FP8)
            nc.vector.tensor_copy(ident_f8[:], ident[:])
            # packed DoubleRow transpose selector: sel[p, o, h*128+y] = (h==o && y==p)
            sel2 = pp.tile([128, 512], FP8)
            nc.vector.memset(sel2[:], 0.0)
            sel2_v = sel2[:].rearrange("p (o h y) -> p o h y", o=2, h=2)
            nc.vector.tensor_copy(sel2_v[:, 0, 0, :], ident_f8[:])
            nc.vector.tensor_copy(sel2_v[:, 1, 1, :], ident_f8[:])

            wg_sb = pp.tile([128, NLAYERS * 128], F32)
            nc.sync.dma_start(
                wg_sb[:].rearrange("p (l f) -> p l f", l=NLAYERS),
                wg_d[:].rearrange("l p f -> p l f"),
            )
            wl_sb = pp.tile([128, NLAYERS * 128], F32)
            nc.sync.dma_start(
                wl_sb[:].rearrange("p (l f) -> p l f", l=NLAYERS),
                wl_d[:].rearrange("l p f -> p l f"),
            )
            bg_sb = pp.tile([128, NLAYERS], F32)
            nc.sync.dma_start(bg_sb[:], bg_d[:].rearrange("l p -> p l"))
            bl_sb = pp.tile([128, NLAYERS], F32)
            nc.sync.dma_start(bl_sb[:], bl_d[:].rearrange("l p -> p l"))
            fcw_sb = pp.tile([128, 4 * 128], F32)
            nc.sync.dma_start(
                fcw_sb[:].rearrange("p (j f) -> p j f", j=4),
                fcw_d[:].rearrange("(j p) f -> p j f", p=128),
            )
            fcb_sb = pp.tile([128, 1], F32)
            nc.sync.dma_start(fcb_sb[:], fcb_d[:].rearrange("(p o) -> p o", o=1))
            c_sb = pp.tile([128, 1], F32)
            nc.sync.dma_start(c_sb[:], c_d[:].rearrange("(p o) -> p o", o=1))
            fow_sb = pp.tile([128, NCLASS], F32)
            nc.sync.dma_start(fow_sb[:], fow_d[:])
            fob_sb = pp.tile([NCLASS, 1], F32)
            nc.sync.dma_start(fob_sb[:], fob_d[:].rearrange("(p o) -> p o", o=1))

            c01 = pp.tile([128, 1], F32)
            nc.vector.tensor_scalar_mul(c01[:], c_sb[:], GAMMA)
            fcw_bf = pp.tile([128, 4 * 128], BF16)
            nc.vector.tensor_copy(fcw_bf[:], fcw_sb[:])

            # M_i = beta_i*wg_i + (1-beta_i)*I; m1 = (1-a)M, m0 = a*M (bf16)
            m1_bf = pp.tile([128, NLAYERS * 128], BF16)
            m0_bf = pp.tile([128, NLAYERS * 128], BF16)
            wl_bf = pp.tile([128, NLAYERS * 128], BF16)
            nc.vector.tensor_copy(wl_bf[:], wl_sb[:])
            for i in range(NLAYERS):
                sl = slice(i * 128, (i + 1) * 128)
                mtmp = stp.tile([128, 128], F32, tag="mtmp")
                nc.vector.tensor_scalar_mul(mtmp[:], wg_sb[:, sl], betas[i])
                mtmp2 = stp.tile([128, 128], F32, tag="mtmp2")
                nc.vector.tensor_scalar_mul(mtmp2[:], ident[:], 1.0 - betas[i])
                nc.vector.tensor_add(mtmp[:], mtmp[:], mtmp2[:])
                nc.vector.tensor_scalar_mul(m1_bf[:, sl], mtmp[:], 1.0 - ALPHA)
                nc.vector.tensor_scalar_mul(m0_bf[:, sl], mtmp[:], ALPHA)

            fow_bf = pp.tile([128, NCLASS], BF16)
            nc.vector.tensor_copy(fow_bf[:], fow_sb[:])

            dinv_row = pp.tile([1, NLOC], F32)
            dinv_nch = pp.tile([128, RB], F32)
            h0_bf = pp.tile([128, NLOC], BF16)

            # transposed-A views: free index = k*1024 + rb*128 + s
            at_q = at_all[:].rearrange(
                "p (k rbp run) -> p k rbp run", k=K, rbp=4, run=256
            )
            at_dr = at_all[:].rearrange(
                "p (kp o rb s) -> p kp o rb s", kp=KP, o=2, rb=RB
            )

            # =============== fc_in (x -> h0^T, bf16) ========================
            def emit_fc():
              with (
                tc.tile_pool(name="fcpool", bufs=2) as fcp,
                tc.tile_pool(name="ps_fc", bufs=2, space="PSUM") as psfc,
              ):
                xt_all = fcp.tile([128, 4 * NLOC], BF16, bufs=1)
                for rb in range(RB):
                    x_chunk = fcp.tile([128, NFP], F32, tag="xch")
                    nc.sync.dma_start(
                        x_chunk[:], x_c[rb * 128 : (rb + 1) * 128, :]
                    )
                    x_stage = fcp.tile([128, NFP], BF16, tag="xbf")
                    nc.vector.tensor_copy(x_stage[:], x_chunk[:])
                    ps_x = psfc.tile([128, 512], F32, tag="psfc")
                    for j in range(4):
                        nc.tensor.matmul(
                            ps_x[:, j * 128 : (j + 1) * 128],
                            x_stage[:, j * 128 : (j + 1) * 128],
                            ident_bf[:],
                            start=(j == 0),
                            stop=(j == 3),
                            skip_group_check=True,
                        )
                    xt_view = xt_all[:].rearrange(
                        "p (j rb s) -> p j rb s", j=4, rb=RB, s=128
                    )
                    nc.vector.tensor_copy(xt_view[:, :, rb : rb + 1, :], ps_x[:])
                for nh in range(2):
                    ps_h = psfc.tile([128, 512], F32, tag="psfc")
                    for j in range(4):
                        nc.tensor.matmul(
                            ps_h[:],
                            fcw_bf[:, j * 128 : (j + 1) * 128],
                            xt_all[:, j * NLOC + nh * 512 : j * NLOC + (nh + 1) * 512],
                            start=(j == 0),
                            stop=(j == 3),
                        )
                    htmp = fcp.tile([128, 512], F32, tag="htmp")
                    nc.scalar.activation(
                        htmp[:],
                        ps_h[:],
                        mybir.ActivationFunctionType.Relu,
                        bias=fcb_sb[:, 0:1],
                    )
                    nc.scalar.activation(
                        h0_bf[:, nh * 512 : (nh + 1) * 512],
                        htmp[:],
                        mybir.ActivationFunctionType.Identity,
                        bias=c01[:, 0:1],
                        scale=1.0 - GAMMA,
                    )

            # ====== pass 0 + layers share pools (layer-0 gathers are emitted
            # inside the pass-0 stream) ======
            with (
                tc.tile_pool(name="apool", bufs=2) as ap_pool,
                tc.tile_pool(name="lpool", bufs=1) as lp,
                tc.tile_pool(name="tmp4", bufs=4) as tp,
                tc.tile_pool(name="ps_p", bufs=2, space="PSUM") as ps_pp,
            ):
                deg_parts = pp.tile([128, 32], F32)
                deg_nch = pp.tile([128, RB], F32)
                psb = [
                    [
                        lp.tile([128, 32 * 128], FP8, name=f"psb{par}_{h}")
                        for h in range(2)
                    ]
                    for par in range(2)
                ]
                plocs = [
                    tp.tile([128, RB * 128], FP8, tag=f"ploc{i % 2}", bufs=1,
                            name=f"ploc{i}")
                    for i in range(NLAYERS)
                ]

                def emit_gather(i, half, hT_src, dq=None):
                    """transpose H half -> node-major fp8 P = dinv*H (scaled
                    on scalar straight out of PSUM), AllGather it, land it in
                    psb[i%2][half]. dq picks the DMA queue for cin/psb moves
                    (gpsimd during pass 0, idle sync queue inside layers)."""
                    dq = dq or nc.gpsimd
                    ploc = plocs[i]
                    cin = dram.tile([NLOC // 2, 128], FP8, name=f"ccin{i}_{half}")
                    cout = dram.tile(
                        [N // 2, 128], FP8, addr_space="Shared",
                        name=f"ccout{i}_{half}",
                    )
                    for nb in range(half * 4, half * 4 + 4):
                        ps_p = ps_pp.tile([128, 128], F32, tag="psp")
                        nc.tensor.matmul(
                            ps_p[:],
                            hT_src[:, nb * 128 : (nb + 1) * 128],
                            ident_bf[:],
                            start=True,
                            stop=True,
                        )
                        nc.scalar.activation(
                            ploc[:, nb * 128 : (nb + 1) * 128],
                            ps_p[:],
                            mybir.ActivationFunctionType.Copy,
                            scale=dinv_nch[:, nb : nb + 1],
                        )
                    dq.dma_start(
                        cin[:].rearrange("(nb p) f -> p nb f", p=128),
                        ploc[:, half * 512 : (half + 1) * 512].rearrange(
                            "p (nb f) -> p nb f", nb=4
                        ),
                    )
                    nc.gpsimd.collective_compute(
                        "AllGather",
                        mybir.AluOpType.bypass,
                        replica_groups=[list(range(NCORES))],
                        ins=[cin[:].opt()],
                        outs=[cout[:].opt()],
                    )
                    dq.dma_start(
                        psb[i % 2][half][:].rearrange(
                            "p (c q f) -> p c q f", c=8, q=4
                        ),
                        cout[:].rearrange("(c q p) f -> p c q f", p=128, q=4),
                    )

                # ---- pass 0: 16 slabs of (2 row-blocks x 2048 cols) ----
                # (rbp-major order: after all four column-quarters of a
                # row-block pair, that pair's degrees are final -> dinv)
                CW = 2048  # slab column width
                NCC = N // CW  # 4 column quarters
                slabs = [(cc, rbp) for rbp in range(RB // 2) for cc in range(NCC)]
                a_f8s = {}

                def slab_load(idx):
                    cc, rbp = slabs[idx]
                    a_pair = ap_pool.tile([128, 2 * CW], F32, tag="apair")
                    nc.sync.dma_start(
                        a_pair[:].rearrange("p (o c) -> p o c", o=2),
                        adj_c[
                            rbp * 256 : (rbp + 1) * 256,
                            cc * CW : (cc + 1) * CW,
                        ].rearrange("(o p) c -> p o c", p=128),
                    )
                    a_f8 = ap_pool.tile([128, 2 * CW], FP8, tag="af8")
                    for o in range(2):
                        dix = (rbp * 2 + o) * NCC + cc
                        nc.scalar.activation(
                            a_f8[:, o * CW : (o + 1) * CW],
                            a_pair[:, o * CW : (o + 1) * CW],
                            mybir.ActivationFunctionType.Copy,
                            accum_out=deg_parts[:, dix : dix + 1],
                        )
                    a_f8s[idx] = a_f8

                def slab_transpose(idx):
                    cc, rbp = slabs[idx]
                    a_f8 = a_f8s.pop(idx)
                    af8_v = a_f8[:].rearrange("p (o c) -> p o c", o=2)
                    sel2_dr = sel2[:].rearrange("p (o y) -> p o y", o=2)
                    for jg in range(CW // 512):
                        ps_tr = ps_trp.tile([128, 1024], F32, tag="pstr")
                        for jj in range(4):
                            j = jg * 4 + jj
                            nc.tensor.matmul(
                                ps_tr[:, jj * 256 : (jj + 1) * 256],
                                af8_v[:, :, j * 128 : (j + 1) * 128],
                                sel2_dr,
                                start=(jj % 2 == 0),
                                stop=(jj % 2 == 1),
                                perf_mode=mybir.MatmulPerfMode.DoubleRow,
                                skip_group_check=True,
                            )
                        k0 = cc * (CW // 128) + jg * 4
                        # dst: 4 chunks x contiguous 256B rb-pair run (2D AP)
                        dst = at_q[:, k0 : k0 + 4, rbp, :]
                        if idx < 14 and jg % 4 == 3:
                            nc.scalar.copy(dst, ps_tr[:])
                        else:
                            nc.vector.tensor_copy(dst, ps_tr[:])

                with tc.tile_pool(name="ps_tr", bufs=2, space="PSUM") as ps_trp:
                    rec_nch = pp.tile([128, RB], F32)
                    deg_rbc = deg_parts[:].rearrange("p (rb cc) -> p rb cc", cc=NCC)
                    for rbp in range(RB // 2):
                        for cc in range(NCC):
                            idx = rbp * NCC + cc
                            slab_load(idx)
                            if idx == 0:
                                slab_transpose(0)
                                # fc_in emitted here: its x DMA queues behind
                                # slab 0, its PE work fills the slab-1 wait
                                emit_fc()
                                continue
                            slab_transpose(idx)
                        # degrees for row blocks (2rbp, 2rbp+1) are final now:
                        # their full 8192 columns have been accumulated.
                        dsl = slice(2 * rbp, 2 * rbp + 2)
                        nc.vector.tensor_reduce(
                            deg_nch[:, dsl],
                            deg_rbc[:, dsl, :],
                            axis=mybir.AxisListType.X,
                            op=mybir.AluOpType.add,
                        )
                        nc.vector.reciprocal(rec_nch[:, dsl], deg_nch[:, dsl])
                        nc.scalar.sqrt(dinv_nch[:, dsl], rec_nch[:, dsl])
                        if rbp == 1:
                            emit_gather(0, 0, h0_bf)
                        elif rbp == 3:
                            dinv_dram = dram.tile([1, NLOC], F32, name="dinv_dram")
                            nc.sync.dma_start(
                                dinv_dram[:].rearrange("o (j p) -> (o p) j", p=128),
                                dinv_nch[:],
                            )
                            nc.sync.dma_start(dinv_row[:], dinv_dram[:])
                            emit_gather(0, 1, h0_bf)

                # ---- layers ----
                with (
                    tc.tile_pool(name="ps_st", bufs=1, space="PSUM") as ps_stp,
                    tc.tile_pool(name="ps_aux", bufs=2, space="PSUM") as ps_auxp,
                ):
                    b_d1 = lp.tile([128, NLOC], F32)
                    ps_b = ps_auxp.tile([128, 512], F32, tag="aux", name="psb_bc0")
                    ps_b2 = ps_auxp.tile([128, 512], F32, tag="aux", name="psb_bc1")
                    for nh, psx in ((0, ps_b), (1, ps_b2)):
                        nc.tensor.matmul(
                            psx[:],
                            ones_row[0:1, :],
                            dinv_row[0:1, nh * 512 : (nh + 1) * 512],
                            start=True,
                            stop=True,
                        )
                        nc.vector.tensor_copy(
                            b_d1[:, nh * 512 : (nh + 1) * 512], psx[:]
                        )

                    hT = h0_bf
                    for i in range(NLAYERS):
                        # ---- SpMM: S^T = P^T @ A_loc^T (fp8 DoubleRow) ----
                        # half h holds global chunks kk = c*8 + h*4 + {0..3},
                        # i.e. chunk pairs kp = c*4 + h*2 + {0,1}.
                        # rh-major SpMM: st[:, 0:512] finishes at half-SpMM so
                        # its epilogue + the next-layer gather overlap rh=1.
                        st = ps_stp.tile([128, NLOC], F32, tag="st", name=f"st_{i}")
                        ah_bf = tp.tile(
                            [128, NLOC], BF16, tag="ahbf", bufs=2, name=f"ah{i}"
                        )
                        hT_new = stp.tile(
                            [128, NLOC], BF16, tag="hT", name=f"hT_l{i + 1}"
                        )
                        for nh in range(2):
                            rh = nh
                            n_mm = 0
                            for half in range(2):
                                psb_v = psb[i % 2][half][:].rearrange(
                                    "p (c j o f) -> p c j o f", c=8, j=2, o=2
                                )
                                for c in range(8):
                                    for j in range(2):
                                        kp = c * 4 + half * 2 + j
                                        lhs_dr = psb_v[:, c, j, :, :]
                                        rhs_dr = at_dr[
                                            :, kp, :, rh * 4 : (rh + 1) * 4, :
                                        ]
                                        nc.tensor.matmul(
                                            st[:, rh * 512 : (rh + 1) * 512],
                                            lhs_dr,
                                            rhs_dr,
                                            start=(n_mm == 0),
                                            stop=(n_mm == KP - 1),
                                            perf_mode=mybir.MatmulPerfMode.DoubleRow,
                                        )
                                        n_mm += 1

                            sl = slice(nh * 512, (nh + 1) * 512)
                            nc.vector.tensor_mul(ah_bf[:, sl], st[:, sl], b_d1[:, sl])
                            ps_l = ps_auxp.tile(
                                [128, 512], F32, tag="aux", name=f"psl{i}_{nh}"
                            )
                            nc.tensor.matmul(
                                ps_l[:],
                                wl_bf[:, i * 128 : (i + 1) * 128],
                                ah_bf[:, sl],
                                start=True,
                                stop=True,
                            )
                            ps_g = ps_auxp.tile(
                                [128, 512], F32, tag="aux", name=f"psg{i}_{nh}"
                            )
                            nc.tensor.matmul(
                                ps_g[:],
                                m1_bf[:, i * 128 : (i + 1) * 128],
                                ah_bf[:, sl],
                                start=True,
                                stop=False,
                            )
                            nc.tensor.matmul(
                                ps_g[:],
                                m0_bf[:, i * 128 : (i + 1) * 128],
                                h0_bf[:, sl],
                                start=False,
                                stop=True,
                            )
                            linv = tp.tile(
                                [128, 512], F32, tag="linv", bufs=2,
                                name=f"lv{i}_{nh}",
                            )
                            nc.scalar.activation(
                                linv[:],
                                ps_l[:],
                                mybir.ActivationFunctionType.Identity,
                                bias=bl_sb[:, i : i + 1],
                            )
                            gc = tp.tile(
                                [128, 512], F32, tag="gc", bufs=2,
                                name=f"gc{i}_{nh}",
                            )
                            nc.scalar.activation(
                                gc[:],
                                ps_g[:],
                                mybir.ActivationFunctionType.Relu,
                                bias=bg_sb[:, i : i + 1],
                            )
                            nc.vector.tensor_add(hT_new[:, sl], linv[:], gc[:])
                            if i < NLAYERS - 1:
                                emit_gather(i + 1, nh, hT_new, dq=nc.sync)
                        hT = hT_new

                    # ---- output head ----
                    ps_o = ps_auxp.tile(
                        [NCLASS, NLOC], F32, tag="auxo", name="pso", bufs=1
                    )
                    for nh in range(2):
                        nc.tensor.matmul(
                            ps_o[:, nh * 512 : (nh + 1) * 512],
                            fow_bf[:, 0:NCLASS],
                            hT[:, nh * 512 : (nh + 1) * 512],
                            start=True,
                            stop=True,
                        )
                    out_sb = lp.tile([NCLASS, NLOC], F32)
                    nc.scalar.activation(
                        out_sb[:],
                        ps_o[:],
                        mybir.ActivationFunctionType.Identity,
                        bias=fob_sb[:, 0:1],
                    )
                    nc.sync.dma_start(out_t[:], out_sb[:])

    nc.compile()
    return nc


_program_cache = {}


def _get_program():
    if "nc" not in _program_cache:
        _program_cache["nc"] = build_program()
    return _program_cache["nc"]


def kernel(
    x,
    adj,
    fc_in_w,
    fc_in_b,
    c,
    w_gcnii,
    b_gcnii,
    w_lin,
    b_lin,
    fc_out_w,
    fc_out_b,
    _trace=False,
):
    x = np.asarray(x, dtype=np.float32)
    adj = np.asarray(adj, dtype=np.float32)
    x_pad = np.zeros((N, NFP), np.float32)
    x_pad[:, :NFEAT] = x
    fcw_pad = np.zeros((NFP, NHID), np.float32)
    fcw_pad[:NFEAT, :] = np.asarray(fc_in_w, np.float32)

    shared = {
        "fc_in_w_p": fcw_pad,
        "fc_in_b": np.asarray(fc_in_b, np.float32),
        "c_vec": np.asarray(c, np.float32),
        "w_gcnii": np.ascontiguousarray(w_gcnii, np.float32),
        "b_gcnii": np.ascontiguousarray(b_gcnii, np.float32),
        "w_lin": np.ascontiguousarray(w_lin, np.float32),
        "b_lin": np.ascontiguousarray(b_lin, np.float32),
        "fc_out_w": np.ascontiguousarray(fc_out_w, np.float32),
        "fc_out_b": np.asarray(fc_out_b, np.float32),
    }
    rr = np.arange(NLOC)
    in_maps = []
    for cix in range(NCORES):
        r0, r1 = cix * NLOC, (cix + 1) * NLOC
        m = dict(shared)
        blk = np.array(adj[r0:r1, :], dtype=np.float32, copy=True)
        blk[rr, r0 + rr] += 1.0  # fold A+I during sharding
        m["adj_c"] = blk
        m["x_c"] = np.ascontiguousarray(x_pad[r0:r1, :])
        in_maps.append(m)

    nc = _get_program()
    res = bass_utils.run_bass_kernel_spmd(
        nc, in_maps=in_maps, core_ids=list(range(NCORES)), trace=_trace
    )
    out = np.empty((N, NCLASS), np.float32)
    for cix in range(NCORES):
        out[cix * NLOC : (cix + 1) * NLOC, :] = res.results[cix]["out_t"].T
    kernel.last_exec_time_ns = res.exec_time_ns
    kernel.last_results = res
    return out


kernel.last_exec_time_ns = None
kernel.last_results = None
